# revision 10
# baseline (speedup 1.0000x reference)
"""Trainium2 Bass kernel for 2D cubic Hermite interpolation (nn_CubicHermite2d).

Math: with x1 = arange(W), x2 = arange(H) (per the problem spec), the whole
op is linear in `signal`:

    result[b, r, q] = sum_{h,w} M2[h, r] * signal[b, h, w] * M1[w, q]

where M1 [W, Nx] / M2 [H, Ny] are 4-banded cubic-Hermite interpolation
matrices built on the host from xs / ys.  Queries are sorted, so contiguous
query groups have source-row bands that fit in a 128-row window -> each
output block is a single K=128 matmul on the PE (no accumulation, no
transposes):

    step 1:  v[g1][wp, r]   = sig[h_lo2:+128, w_lo1:+128].T @ M2[h_lo2:+128, rs:re]
    step 2:  out[b, rm, q]  = v[g1][:, rm*128:+128].T @ M1[w_lo1:+128, qs:qe]

Sharding: data-parallel over batch B=32 across 8 cores (4 batches/core).
"""

import os
import sys

import numpy as np

for _p in ("/root/.axon_site", "/root/.axon_site/_ro/trn_rl_repo",
           "/root/.axon_site/_ro/pypackages", "/opt/trn_rl_repo"):
    if os.path.isdir(_p) and _p not in sys.path:
        sys.path.append(_p)

import concourse.bass as bass
import concourse.mybir as mybir
from concourse import bacc
from concourse.bass_utils import run_bass_kernel_spmd
from concourse.tile import TileContext

# Problem shapes (hardcoded per spec)
B, H, W = 32, 512, 512
NX, NY = 1024, 1024
N_CORES = 8
NB = B // N_CORES  # batches per core

P = 128
F32 = mybir.dt.float32
# float32r: single-pass fp32 matmul (1 cyc/row vs 4 for exact fp32).
USE_F32R = os.environ.get("CH2D_F32R", "1") == "1"
MM_DT = mybir.dt.float32r if USE_F32R else mybir.dt.float32


def _interp_matrix(n, u):
    """[n, Q] float64 matrix M with (y @ M) == cubic-Hermite interp of y at u,
    for grid x = arange(n), matching the reference's searchsorted/slope rules."""
    q = len(u)
    m = np.zeros((n, q), dtype=np.float64)
    idx = np.searchsorted(np.arange(1, n - 1, dtype=np.float64), u.astype(np.float64))
    t = u.astype(np.float64) - idx
    t2, t3 = t * t, t * t * t
    h00 = 1.0 - 3.0 * t2 + 2.0 * t3
    h10 = t - 2.0 * t2 + t3
    h01 = 3.0 * t2 - 2.0 * t3
    h11 = t3 - t2
    cols = np.arange(q)
    for k in range(q):
        i = int(idx[k])
        c = cols[k]
        m[i, c] += h00[k]
        m[i + 1, c] += h01[k]
        if i == 0:
            m[1, c] += h10[k]
            m[0, c] -= h10[k]
        else:
            m[i + 1, c] += h10[k] / 2
            m[i - 1, c] -= h10[k] / 2
        if i + 1 == n - 1:
            m[n - 1, c] += h11[k]
            m[n - 2, c] -= h11[k]
        else:
            m[i + 2, c] += h11[k] / 2
            m[i, c] -= h11[k] / 2
    return m, idx.astype(np.int64)


def _make_groups(idx, n, max_size=512, bank=512):
    """Greedy contiguous query groups; each group's source rows fit in a
    128-row window starting at row_lo.  Groups never cross `bank`-multiples
    in query index (PSUM bank boundary) and have even sizes where possible
    (fp32r ISA: even matmul N + 8B-aligned PSUM offsets).
    Returns [(q_start, q_end, row_lo)]."""
    qn = len(idx)
    lo = np.maximum(idx - 1, 0)
    hi = np.minimum(idx + 2, n - 1)
    groups = []
    s = 0
    while s < qn:
        row_lo = int(lo[s])
        e = s
        while e < qn:
            if hi[e] - row_lo + 1 > P:
                break
            if e - s >= max_size:
                break
            if e > s and (e % bank) == 0:
                break
            e += 1
        if e < qn and (e - s) % 2 == 1 and e - s > 1:
            e -= 1  # keep sizes (and hence starts) even for fp32r
        groups.append((s, e, min(row_lo, n - P)))
        s = e
    return groups


def _groups_f32r_ok(groups):
    """fp32r needs even matmul N and even (8B-aligned) PSUM column offsets."""
    return all(qs % 2 == 0 and (qe - qs) % 2 == 0 for qs, qe, _ in groups)


def _build_nc(g1, g2, mm_dt):
    MM_DT = mm_dt
    nc = bacc.Bacc("TRN2", target_bir_lowering=False,
                   name="cubic_hermite2d", num_devices=N_CORES)
    sig_d = nc.dram_tensor("signal", [NB, H, W], MM_DT, kind="ExternalInput")
    w2_d = nc.dram_tensor("w2p", [P, NY], MM_DT, kind="ExternalInput")
    w1_d = nc.dram_tensor("w1p", [P, NX], MM_DT, kind="ExternalInput")
    out_d = nc.dram_tensor("out", [NB, NY, NX], F32, kind="ExternalOutput")

    copy_i = 0

    with (
        TileContext(nc) as tc,
        tc.tile_pool(name="const", bufs=1) as const_pool,
        tc.tile_pool(name="sig", bufs=2 * len(g2)) as sig_pool,
        tc.tile_pool(name="vbuf", bufs=2 * len(g1)) as v_pool,
        tc.tile_pool(name="obuf", bufs=3) as o_pool,
        tc.tile_pool(name="vps", bufs=2, space="PSUM") as vps_pool,
        tc.tile_pool(name="ops", bufs=2, space="PSUM") as ops_pool,
    ):
        w2_s = const_pool.tile([P, NY], MM_DT, name="w2s")
        nc.sync.dma_start(out=w2_s[:], in_=w2_d[:, :])
        w1_s = const_pool.tile([P, NX], MM_DT, name="w1s")
        nc.sync.dma_start(out=w1_s[:], in_=w1_d[:, :])

        def copy_out(dst, src):
            # alternate PSUM->SBUF copies between DVE and ACT to split the load
            nonlocal copy_i
            if copy_i % 2 == 0:
                nc.vector.tensor_copy(out=dst, in_=src)
            else:
                nc.scalar.copy(out=dst, in_=src)
            copy_i += 1

        for b in range(NB):
            sig_tiles = []
            for (_, _, hlo) in g2:
                st = sig_pool.tile([P, W], MM_DT, name="sigt")
                nc.sync.dma_start(out=st[:], in_=sig_d[b, hlo:hlo + P, :])
                sig_tiles.append(st)

            v_tiles = []
            for (qs1, qe1, wlo) in g1:
                vps = vps_pool.tile([P, NY], F32, name="vps")
                for gi2, (rs, re, _) in enumerate(g2):
                    nc.tensor.matmul(
                        out=vps[:, rs:re],
                        lhsT=sig_tiles[gi2][:, wlo:wlo + P],
                        rhs=w2_s[:, rs:re],
                        start=True, stop=True,
                    )
                vt = v_pool.tile([P, NY], MM_DT, name="vt")
                copy_out(vt[:], vps[:])
                v_tiles.append(vt)

            for mi in range(NY // P):
                ops = ops_pool.tile([P, NX], F32, name="ops")
                for gi1, (qs, qe, _) in enumerate(g1):
                    nc.tensor.matmul(
                        out=ops[:, qs:qe],
                        lhsT=v_tiles[gi1][:, mi * P:(mi + 1) * P],
                        rhs=w1_s[:, qs:qe],
                        start=True, stop=True,
                    )
                ot = o_pool.tile([P, NX], F32, name="ot")
                copy_out(ot[:], ops[:])
                nc.sync.dma_start(out=out_d[b, mi * P:(mi + 1) * P, :], in_=ot[:])

    nc.compile()
    return nc


def _prepare(signal, x1, x2, xs, ys):
    """Host-side prep: sorted-order permutations, interp matrices, groups."""
    xs = np.asarray(xs, dtype=np.float32)
    ys = np.asarray(ys, dtype=np.float32)
    perm_x = None
    if np.any(np.diff(xs) < 0):
        perm_x = np.argsort(xs, kind="stable")
        xs = xs[perm_x]
    perm_y = None
    if np.any(np.diff(ys) < 0):
        perm_y = np.argsort(ys, kind="stable")
        ys = ys[perm_y]

    m1, i1 = _interp_matrix(W, xs)
    m2, i2 = _interp_matrix(H, ys)
    g1 = _make_groups(i1, W)
    g2 = _make_groups(i2, H)

    # pack band blocks: rows = the group's 128-row source window
    w1p = np.zeros((P, NX), dtype=np.float32)
    for (qs, qe, wlo) in g1:
        w1p[:, qs:qe] = m1[wlo:wlo + P, qs:qe]
    w2p = np.zeros((P, NY), dtype=np.float32)
    for (rs, re, hlo) in g2:
        w2p[:, rs:re] = m2[hlo:hlo + P, rs:re]
    return g1, g2, w1p, w2p, perm_x, perm_y


_NC_CACHE = {}


def _run(inputs, trace=False, trace_kwargs=None):
    signal = np.ascontiguousarray(np.asarray(inputs["signal"], dtype=np.float32))
    g1, g2, w1p, w2p, perm_x, perm_y = _prepare(
        signal, inputs["x1"], inputs["x2"], inputs["xs"], inputs["ys"])

    use_f32r = USE_F32R and _groups_f32r_ok(g1) and _groups_f32r_ok(g2)
    mm_dt = mybir.dt.float32r if use_f32r else mybir.dt.float32
    key = (tuple(g1), tuple(g2), mm_dt)
    nc = _NC_CACHE.get(key)
    if nc is None:
        nc = _build_nc(g1, g2, mm_dt)
        _NC_CACHE[key] = nc

    in_maps = []
    for c in range(N_CORES):
        in_maps.append({
            "signal": np.ascontiguousarray(signal[c * NB:(c + 1) * NB]),
            "w2p": w2p,
            "w1p": w1p,
        })
    res = run_bass_kernel_spmd(
        nc, in_maps, core_ids=list(range(N_CORES)),
        trace=trace, **(trace_kwargs or {}),
    )
    out = np.concatenate([r["out"] for r in res.results], axis=0)

    # restore original (unsorted) query order if needed
    if perm_y is not None:
        inv = np.empty_like(perm_y)
        inv[perm_y] = np.arange(len(perm_y))
        out = out[:, inv, :]
    if perm_x is not None:
        inv = np.empty_like(perm_x)
        inv[perm_x] = np.arange(len(perm_x))
        out = out[:, :, inv]
    return out, res


def kernel(signal, x1, x2, xs, ys):
    out, _ = _run({"signal": signal, "x1": x1, "x2": x2, "xs": xs, "ys": ys})
    return out


# revision 14
# speedup vs baseline: 1.3686x; 1.3686x over previous
"""Trainium2 Bass kernel for 2D cubic Hermite interpolation (nn_CubicHermite2d).

Math: with x1 = arange(W), x2 = arange(H) (per the problem spec), the whole
op is linear in `signal`:

    result[b, r, q] = sum_{h,w} M2[h, r] * signal[b, h, w] * M1[w, q]

where M1 [W, Nx] / M2 [H, Ny] are 4-banded cubic-Hermite interpolation
matrices built on the host from xs / ys.  Queries are sorted, so contiguous
query groups have source-row bands that fit in a 128-row window -> each
output block is a single K=128 matmul on the PE (no accumulation, no
transposes):

    step 1:  v[g1][wp, r]   = sig[h_lo2:+128, w_lo1:+128].T @ M2[h_lo2:+128, rs:re]
    step 2:  out[b, rm, q]  = v[g1][:, rm*128:+128].T @ M1[w_lo1:+128, qs:qe]

Sharding: data-parallel over batch B=32 across 8 cores (4 batches/core).
"""

import os
import sys

import numpy as np

for _p in ("/root/.axon_site", "/root/.axon_site/_ro/trn_rl_repo",
           "/root/.axon_site/_ro/pypackages", "/opt/trn_rl_repo"):
    if os.path.isdir(_p) and _p not in sys.path:
        sys.path.append(_p)

import concourse.bass as bass
import concourse.mybir as mybir
from concourse import bacc
from concourse.bass_utils import run_bass_kernel_spmd
from concourse.tile import TileContext

# Problem shapes (hardcoded per spec)
B, H, W = 32, 512, 512
NX, NY = 1024, 1024
N_CORES = 8
NB = B // N_CORES  # batches per core

P = 128
F32 = mybir.dt.float32
# float32r: single-pass fp32 matmul (1 cyc/row vs 4 for exact fp32).
USE_F32R = os.environ.get("CH2D_F32R", "1") == "1"
MM_DT = mybir.dt.float32r if USE_F32R else mybir.dt.float32


def _interp_matrix(n, u):
    """[n, Q] float64 matrix M with (y @ M) == cubic-Hermite interp of y at u,
    for grid x = arange(n), matching the reference's searchsorted/slope rules."""
    q = len(u)
    m = np.zeros((n, q), dtype=np.float64)
    idx = np.searchsorted(np.arange(1, n - 1, dtype=np.float64), u.astype(np.float64))
    t = u.astype(np.float64) - idx
    t2, t3 = t * t, t * t * t
    h00 = 1.0 - 3.0 * t2 + 2.0 * t3
    h10 = t - 2.0 * t2 + t3
    h01 = 3.0 * t2 - 2.0 * t3
    h11 = t3 - t2
    cols = np.arange(q)
    for k in range(q):
        i = int(idx[k])
        c = cols[k]
        m[i, c] += h00[k]
        m[i + 1, c] += h01[k]
        if i == 0:
            m[1, c] += h10[k]
            m[0, c] -= h10[k]
        else:
            m[i + 1, c] += h10[k] / 2
            m[i - 1, c] -= h10[k] / 2
        if i + 1 == n - 1:
            m[n - 1, c] += h11[k]
            m[n - 2, c] -= h11[k]
        else:
            m[i + 2, c] += h11[k] / 2
            m[i, c] -= h11[k] / 2
    return m, idx.astype(np.int64)


def _make_groups(idx, n, max_size=512, bank=512):
    """Greedy contiguous query groups; each group's source rows fit in a
    128-row window starting at row_lo.  Groups never cross `bank`-multiples
    in query index (PSUM bank boundary) and have even sizes where possible
    (fp32r ISA: even matmul N + 8B-aligned PSUM offsets).
    Returns [(q_start, q_end, row_lo)]."""
    qn = len(idx)
    lo = np.maximum(idx - 1, 0)
    hi = np.minimum(idx + 2, n - 1)
    groups = []
    s = 0
    while s < qn:
        row_lo = int(lo[s])
        e = s
        while e < qn:
            if hi[e] - row_lo + 1 > P:
                break
            if e - s >= max_size:
                break
            if e > s and (e % bank) == 0:
                break
            e += 1
        if e < qn and (e - s) % 2 == 1 and e - s > 1:
            e -= 1  # keep sizes (and hence starts) even for fp32r
        groups.append((s, e, min(row_lo, n - P)))
        s = e
    return groups


def _groups_f32r_ok(groups):
    """fp32r needs even matmul N and even (8B-aligned) PSUM column offsets."""
    return all(qs % 2 == 0 and (qe - qs) % 2 == 0 for qs, qe, _ in groups)


def _build_nc(g1, g2, mm_dt):
    MM_DT = mm_dt
    nc = bacc.Bacc("TRN2", target_bir_lowering=False,
                   name="cubic_hermite2d", num_devices=N_CORES)
    sig_d = nc.dram_tensor("signal", [NB, H, W], MM_DT, kind="ExternalInput")
    w2_d = nc.dram_tensor("w2p", [P, NY], MM_DT, kind="ExternalInput")
    w1_d = nc.dram_tensor("w1p", [P, NX], MM_DT, kind="ExternalInput")
    out_d = nc.dram_tensor("out", [NB, NY, NX], F32, kind="ExternalOutput")

    copy_i = 0
    # split each 1024-wide output range into per-bank (512) halves so PSUM
    # tiles are single-bank and pools pipeline at bank granularity
    half1 = [[(i, g) for i, g in enumerate(g1) if g[1] <= NX // 2],
             [(i, g) for i, g in enumerate(g1) if g[0] >= NX // 2]]
    half2 = [[(i, g) for i, g in enumerate(g2) if g[1] <= NY // 2],
             [(i, g) for i, g in enumerate(g2) if g[0] >= NY // 2]]
    assert sum(len(h) for h in half1) == len(g1)
    assert sum(len(h) for h in half2) == len(g2)

    with (
        TileContext(nc) as tc,
        tc.tile_pool(name="const", bufs=1) as const_pool,
        tc.tile_pool(name="sig", bufs=NB * len(g2)) as sig_pool,
        tc.tile_pool(name="vbuf", bufs=2 * len(g1)) as v_pool,
        tc.tile_pool(name="obuf", bufs=8) as o_pool,
        tc.tile_pool(name="vps", bufs=4, space="PSUM") as vps_pool,
        tc.tile_pool(name="ops", bufs=4, space="PSUM") as ops_pool,
    ):
        w2_s = const_pool.tile([P, NY], MM_DT, name="w2s")
        nc.sync.dma_start(out=w2_s[:], in_=w2_d[:, :])
        w1_s = const_pool.tile([P, NX], MM_DT, name="w1s")
        nc.sync.dma_start(out=w1_s[:], in_=w1_d[:, :])

        def copy_out(dst, src):
            # alternate PSUM->SBUF copies between DVE and ACT to split the load
            nonlocal copy_i
            if copy_i % 2 == 0:
                nc.vector.tensor_copy(out=dst, in_=src)
            else:
                nc.scalar.copy(out=dst, in_=src)
            copy_i += 1

        # preload every batch's signal tiles up front so later batches'
        # loads are not queued behind earlier batches' stores
        sig_tiles = {}
        for b in range(NB):
            for gi2, (_, _, hlo) in enumerate(g2):
                st = sig_pool.tile([P, W], MM_DT, name="sigt")
                nc.sync.dma_start(out=st[:], in_=sig_d[b, hlo:hlo + P, :])
                sig_tiles[(b, gi2)] = st

        for b in range(NB):
            v_tiles = []
            for (qs1, qe1, wlo) in g1:
                vt = v_pool.tile([P, NY], MM_DT, name="vt")
                for hi, hgroups in enumerate(half2):
                    if not hgroups:
                        continue
                    base = hi * (NY // 2)
                    vps = vps_pool.tile([P, NY // 2], F32, name="vps")
                    for gi2, (rs, re, _) in hgroups:
                        nc.tensor.matmul(
                            out=vps[:, rs - base:re - base],
                            lhsT=sig_tiles[(b, gi2)][:, wlo:wlo + P],
                            rhs=w2_s[:, rs:re],
                            start=True, stop=True,
                        )
                    copy_out(vt[:, base:base + NY // 2], vps[:])
                v_tiles.append(vt)

            for mi in range(NY // P):
                ot = o_pool.tile([P, NX], F32, name="ot")
                for hi, hgroups in enumerate(half1):
                    if not hgroups:
                        continue
                    base = hi * (NX // 2)
                    ops = ops_pool.tile([P, NX // 2], F32, name="ops")
                    for gi1, (qs, qe, _) in hgroups:
                        nc.tensor.matmul(
                            out=ops[:, qs - base:qe - base],
                            lhsT=v_tiles[gi1][:, mi * P:(mi + 1) * P],
                            rhs=w1_s[:, qs:qe],
                            start=True, stop=True,
                        )
                    copy_out(ot[:, base:base + NX // 2], ops[:])
                nc.sync.dma_start(out=out_d[b, mi * P:(mi + 1) * P, :], in_=ot[:])

    nc.compile()
    return nc


def _prepare(signal, x1, x2, xs, ys):
    """Host-side prep: sorted-order permutations, interp matrices, groups."""
    xs = np.asarray(xs, dtype=np.float32)
    ys = np.asarray(ys, dtype=np.float32)
    perm_x = None
    if np.any(np.diff(xs) < 0):
        perm_x = np.argsort(xs, kind="stable")
        xs = xs[perm_x]
    perm_y = None
    if np.any(np.diff(ys) < 0):
        perm_y = np.argsort(ys, kind="stable")
        ys = ys[perm_y]

    m1, i1 = _interp_matrix(W, xs)
    m2, i2 = _interp_matrix(H, ys)
    g1 = _make_groups(i1, W)
    g2 = _make_groups(i2, H)

    # pack band blocks: rows = the group's 128-row source window
    w1p = np.zeros((P, NX), dtype=np.float32)
    for (qs, qe, wlo) in g1:
        w1p[:, qs:qe] = m1[wlo:wlo + P, qs:qe]
    w2p = np.zeros((P, NY), dtype=np.float32)
    for (rs, re, hlo) in g2:
        w2p[:, rs:re] = m2[hlo:hlo + P, rs:re]
    return g1, g2, w1p, w2p, perm_x, perm_y


_NC_CACHE = {}


def _run(inputs, trace=False, trace_kwargs=None):
    signal = np.ascontiguousarray(np.asarray(inputs["signal"], dtype=np.float32))
    g1, g2, w1p, w2p, perm_x, perm_y = _prepare(
        signal, inputs["x1"], inputs["x2"], inputs["xs"], inputs["ys"])

    use_f32r = USE_F32R and _groups_f32r_ok(g1) and _groups_f32r_ok(g2)
    mm_dt = mybir.dt.float32r if use_f32r else mybir.dt.float32
    key = (tuple(g1), tuple(g2), mm_dt)
    nc = _NC_CACHE.get(key)
    if nc is None:
        nc = _build_nc(g1, g2, mm_dt)
        _NC_CACHE[key] = nc

    in_maps = []
    for c in range(N_CORES):
        in_maps.append({
            "signal": np.ascontiguousarray(signal[c * NB:(c + 1) * NB]),
            "w2p": w2p,
            "w1p": w1p,
        })
    res = run_bass_kernel_spmd(
        nc, in_maps, core_ids=list(range(N_CORES)),
        trace=trace, **(trace_kwargs or {}),
    )
    out = np.concatenate([r["out"] for r in res.results], axis=0)

    # restore original (unsorted) query order if needed
    if perm_y is not None:
        inv = np.empty_like(perm_y)
        inv[perm_y] = np.arange(len(perm_y))
        out = out[:, inv, :]
    if perm_x is not None:
        inv = np.empty_like(perm_x)
        inv[perm_x] = np.arange(len(perm_x))
        out = out[:, :, inv]
    return out, res


def kernel(signal, x1, x2, xs, ys):
    out, _ = _run({"signal": signal, "x1": x1, "x2": x2, "xs": xs, "ys": ys})
    return out
